# revision 20
# baseline (speedup 1.0000x reference)
"""BoxRenderLoss Trainium2 kernel.

loss = mean over (box, fragment) pairs of masked min-squared-distance between
each box's 10x10 fragment grid and the other box's 100-point sampled boundary,
both directions, / (2*B*FP).

Algorithm: the min over the 100 boundary points decomposes into the 4 box
edges; each edge's 25-point uniform grid min has the closed form
k* = clamp(round(u/s), 0, 24), val = u - s*k*.  Per (row, fragment) item:
  dmin = min( min(ux^2, vx^2) + valy^2,  min(uy^2, vy^2) + valx^2 )
  mask = min(ux, vx, uy, vy) < 0         (fragment outside other box)
  contribution = dmin * mask

Device layout: partitions = 100 fragment points, free dim = virtual rows
(4096 boxes x 2 directions, data-parallel over 8 cores -> 1024 columns/core,
2 chunks of 512, x|y packed side by side -> FD 1024).  The affine maps
U = gx*w + dx, V = -gx*w + dvx, T = gx*(w*rix) + dx*rix and broadcast SB = s
are K<=2 outer-product matmuls on the TensorEngine.  All matmul operands live
in ONE SBUF tile (rows 0-1 / 32-33 / 64-65 for the base-partition-0/32/64
groups) and each PSUM tensor is consumed by exactly one engine -- PE Matmult
instructions only support a single semaphore wait.  Final per-partition row
sums come free via scalar_tensor_tensor's accum_out; host sums 100x2x8
partials and divides.
"""

import os
import numpy as np

# Exact float32 bit patterns of jnp.linspace(0.0, 1.0, 10) (fragment grid).
_LIN10 = np.array(
    [0, 1038323257, 1046711865, 1051372203, 1055100473,
     1057896676, 1059760811, 1061624946, 1063489081, 1065353216],
    dtype=np.uint32,
).view(np.float32)

_B = 4096
_FP = 100
_N_CORES = 8
_BOX_PER_CORE = _B // _N_CORES          # 512
_COLS = 2 * _BOX_PER_CORE               # 1024 virtual rows per core
_CHUNK = 512
_N_CHUNKS = _COLS // _CHUNK             # 2
_MAGIC = 8388608.0                      # 2^23 round-to-nearest trick

# mm-input tile column layout: [lhsT 0:128 | block0 | block1 | block2]
_LW = 128
_MMW = _LW + 3 * _COLS                  # 3200

LAST_RESULTS = None  # BassKernelResults of the most recent run (for test.py)

_compiled = {}


def _build_nc():
    import concourse.bass as bass
    import concourse.bacc as bacc
    import concourse.tile as tile
    from concourse import mybir

    f32 = mybir.dt.float32
    Op = mybir.AluOpType
    Act = mybir.ActivationFunctionType

    nc = bacc.Bacc("TRN2", target_bir_lowering=False, debug=False,
                   num_devices=_N_CORES)
    f32r = mybir.dt.float32r
    mmin_d = nc.dram_tensor("mmin", [6, _MMW], f32r,
                            kind="ExternalInput").ap()
    out_d = nc.dram_tensor("out", [_FP, _N_CHUNKS], f32,
                           kind="ExternalOutput").ap()

    def blk(b, c):  # rhs slice columns for block b, chunk c
        start = _LW + b * _COLS + c * _CHUNK
        return slice(start, start + _CHUNK)

    from concourse.tile import add_dep_helper

    with tile.TileContext(nc) as tc:
        with (
            tc.tile_pool(name="const", bufs=1) as const,
            tc.tile_pool(name="sb", bufs=3) as sb,
            tc.tile_pool(name="ps", bufs=1, space="PSUM") as ps,
        ):
            mt = const.tile([66, _MMW], f32r)
            # One DMA per base-partition row group so every matmul carries at
            # most one new DMA wait.
            nc.sync.dma_start(mt[0:1, :], mmin_d[0:1, :])
            nc.sync.dma_start(mt[1:2, :], mmin_d[1:2, :])
            nc.sync.dma_start(mt[32:33, :], mmin_d[2:3, :])
            nc.sync.dma_start(mt[33:34, :], mmin_d[3:4, :])
            nc.sync.dma_start(mt[64:65, :], mmin_d[4:5, :])
            nc.sync.dma_start(mt[65:66, :], mmin_d[5:6, :])
            part = const.tile([_FP, _N_CHUNKS], f32)

            # float32r (same bits as f32): makes tile_legalize split each
            # Matmult into LdWeights + Matmult so semaphore waits spread
            # across two PE instructions (Matmult's LW slot fits only one).
            mtr = mt
            gx = mtr[0:2, :_FP]     # [gx; ones]
            gy = mtr[32:34, :_FP]   # [gy; ones]
            one = mtr[64:65, :_FP]  # [ones]

            for c in range(_N_CHUNKS):
                W = 2 * _CHUNK
                U = ps.tile([_FP, W], f32, tag="U")
                V = ps.tile([_FP, W], f32, tag="V")
                T = ps.tile([_FP, W], f32, tag="T")
                SB = ps.tile([_FP, W], f32, tag="SB")
                xh = slice(0, _CHUNK)
                yh = slice(_CHUNK, W)

                nc.tensor.matmul(U[:, xh], gx, mtr[0:2, blk(0, c)])
                nc.tensor.matmul(U[:, yh], gy, mtr[32:34, blk(0, c)])
                nc.tensor.matmul(V[:, xh], gx, mtr[0:2, blk(1, c)])
                nc.tensor.matmul(V[:, yh], gy, mtr[32:34, blk(1, c)])
                nc.tensor.matmul(T[:, xh], gx, mtr[0:2, blk(2, c)])
                nc.tensor.matmul(T[:, yh], gy, mtr[32:34, blk(2, c)])
                sxs = slice(_LW + c * _CHUNK, _LW + c * _CHUNK + _CHUNK)
                sys_ = slice(_LW + _COLS + c * _CHUNK,
                             _LW + _COLS + c * _CHUNK + _CHUNK)
                nc.tensor.matmul(SB[:, xh], one, mtr[64:65, sxs])
                nc.tensor.matmul(SB[:, yh], one, mtr[64:65, sys_])

                usq = sb.tile([_FP, W], f32, tag="usq")
                nc.scalar.activation(usq[:], U[:], Act.Square)
                vs = sb.tile([_FP, W], f32, tag="vs")
                nc.scalar.activation(vs[:], V[:], Act.Copy)
                vsq = sb.tile([_FP, W], f32, tag="vsq")
                nc.scalar.activation(vsq[:], V[:], Act.Square)

                r1 = sb.tile([_FP, W], f32, tag="r1")
                nc.scalar.activation(r1[:], T[:], Act.Relu)
                kc1 = sb.tile([_FP, W], f32, tag="kc1")
                nc.vector.tensor_scalar(kc1[:], r1[:], _MAGIC,
                                        _MAGIC + 24.0, Op.add, Op.min)
                kc2 = sb.tile([_FP, W], f32, tag="kc2")
                nc.scalar.activation(kc2[:], kc1[:], Act.Copy, bias=-_MAGIC)
                sk = sb.tile([_FP, W], f32, tag="sk")
                nc.vector.tensor_tensor(sk[:], kc2[:], SB[:], Op.mult)
                val = sb.tile([_FP, W], f32, tag="val")
                nc.vector.tensor_tensor(val[:], U[:], sk[:], Op.subtract)
                vq = sb.tile([_FP, W], f32, tag="vq")
                nc.scalar.activation(vq[:], val[:], Act.Square)

                m1 = sb.tile([_FP, W], f32, tag="m1")
                nc.vector.tensor_tensor(m1[:], U[:], vs[:], Op.min)
                mm = sb.tile([_FP, _CHUNK], f32, tag="mm")
                nc.vector.tensor_tensor(mm[:], m1[:, xh], m1[:, yh], Op.min)

                exy = sb.tile([_FP, W], f32, tag="exy")
                nc.vector.tensor_tensor(exy[:], usq[:], vsq[:], Op.min)
                e1 = sb.tile([_FP, _CHUNK], f32, tag="e1")
                nc.vector.tensor_tensor(e1[:], exy[:, xh], vq[:, yh], Op.add)
                e2 = sb.tile([_FP, _CHUNK], f32, tag="e2")
                nc.vector.tensor_tensor(e2[:], exy[:, yh], vq[:, xh], Op.add)
                dmin = sb.tile([_FP, _CHUNK], f32, tag="dmin")
                nc.vector.tensor_tensor(dmin[:], e1[:], e2[:], Op.min)

                scr = sb.tile([_FP, _CHUNK], f32, tag="scr")
                nc.vector.scalar_tensor_tensor(
                    scr[:], mm[:], 0.0, dmin[:], Op.is_lt, Op.mult,
                    accum_out=part[:, c:c + 1])

            nc.sync.dma_start(out_d[:], part[:])
    nc.compile()
    return nc


def _combo_cols(A, T):
    """Per-row combo vectors (f32) for fragment-source boxes A vs grid
    boxes T.  Returns dict of [N] arrays."""
    A = A.astype(np.float32, copy=False)
    T = T.astype(np.float32, copy=False)
    w = A[:, 2] - A[:, 0]
    h = A[:, 3] - A[:, 1]
    tw = T[:, 2] - T[:, 0]
    th = T[:, 3] - T[:, 1]
    dx = A[:, 0] - T[:, 0]
    dy = A[:, 1] - T[:, 1]
    dvx = T[:, 2] - A[:, 0]
    dvy = T[:, 3] - A[:, 1]
    with np.errstate(divide="ignore"):
        rix = np.where(tw != 0, np.float32(24.0) / tw, np.float32(0.0))
        riy = np.where(th != 0, np.float32(24.0) / th, np.float32(0.0))
    rix = rix.astype(np.float32)
    riy = riy.astype(np.float32)
    return dict(
        w=w, dx=dx, nw=-w, dvx=dvx, wrx=w * rix, dxrx=dx * rix,
        h=h, dy=dy, nh=-h, dvy=dvy, wry=h * riy, dyry=dy * riy,
        sx=tw / np.float32(24.0), sy=th / np.float32(24.0))


def _mmin_for_core(boxes_c, targets_c):
    """Build the [6, _MMW] matmul-input DRAM tensor for one core."""
    d1 = _combo_cols(boxes_c, targets_c)     # dir1: cols 0:512
    d2 = _combo_cols(targets_c, boxes_c)     # dir2: cols 512:1024
    cat = {k: np.concatenate([d1[k], d2[k]]) for k in d1}

    m = np.zeros((6, _MMW), dtype=np.float32)
    m[0, :_FP] = np.repeat(_LIN10, 10)       # gx  (i of f = i*10+j)
    m[1, :_FP] = 1.0
    m[2, :_FP] = np.tile(_LIN10, 10)         # gy
    m[3, :_FP] = 1.0
    m[4, :_FP] = 1.0                         # ones (K=1 lhsT)
    B0, B1, B2 = (slice(_LW + b * _COLS, _LW + (b + 1) * _COLS)
                  for b in range(3))
    m[0, B0] = cat["w"];    m[1, B0] = cat["dx"]
    m[0, B1] = cat["nw"];   m[1, B1] = cat["dvx"]
    m[0, B2] = cat["wrx"];  m[1, B2] = cat["dxrx"]
    m[2, B0] = cat["h"];    m[3, B0] = cat["dy"]
    m[2, B1] = cat["nh"];   m[3, B1] = cat["dvy"]
    m[2, B2] = cat["wry"];  m[3, B2] = cat["dyry"]
    m[4, _LW:_LW + _COLS] = cat["sx"]
    m[4, _LW + _COLS:_LW + 2 * _COLS] = cat["sy"]
    return m


def kernel(boxes: np.ndarray, targets: np.ndarray) -> np.ndarray:
    from concourse.bass_utils import run_bass_kernel_spmd

    global LAST_RESULTS
    boxes = np.ascontiguousarray(boxes, dtype=np.float32)
    targets = np.ascontiguousarray(targets, dtype=np.float32)
    assert boxes.shape == (_B, 4) and targets.shape == (_B, 4)

    if "nc" not in _compiled:
        _compiled["nc"] = _build_nc()
    nc = _compiled["nc"]

    in_maps = []
    for c in range(_N_CORES):
        rows = slice(c * _BOX_PER_CORE, (c + 1) * _BOX_PER_CORE)
        in_maps.append({"mmin": _mmin_for_core(boxes[rows], targets[rows])})

    trace = bool(int(os.environ.get("BOXLOSS_TRACE", "0")))
    res = run_bass_kernel_spmd(nc, in_maps, list(range(_N_CORES)),
                               trace=trace)
    LAST_RESULTS = res

    total = np.float64(0.0)
    for r in res.results:
        total += r["out"].astype(np.float64).sum()
    loss = total / (2.0 * _B * _FP)
    return np.array(loss, dtype=np.float32)


# revision 21
# speedup vs baseline: 1.0112x; 1.0112x over previous
"""BoxRenderLoss Trainium2 kernel.

loss = mean over (box, fragment) pairs of masked min-squared-distance between
each box's 10x10 fragment grid and the other box's 100-point sampled boundary,
both directions, / (2*B*FP).

Algorithm: the min over the 100 boundary points decomposes into the 4 box
edges; each edge's 25-point uniform grid min has the closed form
k* = clamp(round(u/s), 0, 24), val = u - s*k*.  Per (row, fragment) item:
  dmin = min( min(ux^2, vx^2) + valy^2,  min(uy^2, vy^2) + valx^2 )
  mask = min(ux, vx, uy, vy) < 0         (fragment outside other box)
  contribution = dmin * mask

Device layout: partitions = 100 fragment points, free dim = virtual rows
(4096 boxes x 2 directions, data-parallel over 8 cores -> 1024 columns/core,
2 chunks of 512, x|y packed side by side -> FD 1024).  The affine maps
U = gx*w + dx, V = -gx*w + dvx, T = gx*(w*rix) + dx*rix and broadcast SB = s
are K<=2 outer-product matmuls on the TensorEngine.  All matmul operands live
in ONE SBUF tile (rows 0-1 / 32-33 / 64-65 for the base-partition-0/32/64
groups) and each PSUM tensor is consumed by exactly one engine -- PE Matmult
instructions only support a single semaphore wait.  Final per-partition row
sums come free via scalar_tensor_tensor's accum_out; host sums 100x2x8
partials and divides.
"""

import os
import numpy as np

# Exact float32 bit patterns of jnp.linspace(0.0, 1.0, 10) (fragment grid).
_LIN10 = np.array(
    [0, 1038323257, 1046711865, 1051372203, 1055100473,
     1057896676, 1059760811, 1061624946, 1063489081, 1065353216],
    dtype=np.uint32,
).view(np.float32)

_B = 4096
_FP = 100
_N_CORES = 8
_BOX_PER_CORE = _B // _N_CORES          # 512
_COLS = 2 * _BOX_PER_CORE               # 1024 virtual rows per core
_CHUNK = 512
_N_CHUNKS = _COLS // _CHUNK             # 2
_MAGIC = 8388608.0                      # 2^23 round-to-nearest trick

# mm-input tile column layout: [lhsT 0:128 | block0 | block1 | block2]
_LW = 128
_MMW = _LW + 3 * _COLS                  # 3200

LAST_RESULTS = None  # BassKernelResults of the most recent run (for test.py)

_compiled = {}


def _build_nc():
    import concourse.bass as bass
    import concourse.bacc as bacc
    import concourse.tile as tile
    from concourse import mybir

    f32 = mybir.dt.float32
    Op = mybir.AluOpType
    Act = mybir.ActivationFunctionType

    nc = bacc.Bacc("TRN2", target_bir_lowering=False, debug=False,
                   num_devices=_N_CORES)
    f32r = mybir.dt.float32r
    mmin_d = nc.dram_tensor("mmin", [6, _MMW], f32r,
                            kind="ExternalInput").ap()
    out_d = nc.dram_tensor("out", [_FP, _N_CHUNKS], f32,
                           kind="ExternalOutput").ap()

    def blk(b, c):  # rhs slice columns for block b, chunk c
        start = _LW + b * _COLS + c * _CHUNK
        return slice(start, start + _CHUNK)

    from concourse.tile import add_dep_helper

    with tile.TileContext(nc) as tc:
        with (
            tc.tile_pool(name="const", bufs=1) as const,
            tc.tile_pool(name="sb", bufs=3) as sb,
            tc.tile_pool(name="ps", bufs=1, space="PSUM") as ps,
        ):
            mt = const.tile([66, _MMW], f32r)
            # One DMA per base-partition row group so every matmul carries at
            # most one new DMA wait.
            nc.sync.dma_start(mt[0:2, :], mmin_d[0:2, :])
            nc.sync.dma_start(mt[32:34, :], mmin_d[2:4, :])
            nc.sync.dma_start(mt[64:66, :], mmin_d[4:6, :])
            part = const.tile([_FP, _N_CHUNKS], f32)

            # float32r (same bits as f32): makes tile_legalize split each
            # Matmult into LdWeights + Matmult so semaphore waits spread
            # across two PE instructions (Matmult's LW slot fits only one).
            mtr = mt
            gx = mtr[0:2, :_FP]     # [gx; ones]
            gy = mtr[32:34, :_FP]   # [gy; ones]
            one = mtr[64:65, :_FP]  # [ones]

            for c in range(_N_CHUNKS):
                W = 2 * _CHUNK
                U = ps.tile([_FP, W], f32, tag="U")
                V = ps.tile([_FP, W], f32, tag="V")
                T = ps.tile([_FP, W], f32, tag="T")
                SB = ps.tile([_FP, W], f32, tag="SB")
                xh = slice(0, _CHUNK)
                yh = slice(_CHUNK, W)

                nc.tensor.matmul(U[:, xh], gx, mtr[0:2, blk(0, c)])
                nc.tensor.matmul(U[:, yh], gy, mtr[32:34, blk(0, c)])
                nc.tensor.matmul(V[:, xh], gx, mtr[0:2, blk(1, c)])
                nc.tensor.matmul(V[:, yh], gy, mtr[32:34, blk(1, c)])
                nc.tensor.matmul(T[:, xh], gx, mtr[0:2, blk(2, c)])
                nc.tensor.matmul(T[:, yh], gy, mtr[32:34, blk(2, c)])
                sxs = slice(_LW + c * _CHUNK, _LW + c * _CHUNK + _CHUNK)
                sys_ = slice(_LW + _COLS + c * _CHUNK,
                             _LW + _COLS + c * _CHUNK + _CHUNK)
                nc.tensor.matmul(SB[:, xh], one, mtr[64:65, sxs])
                nc.tensor.matmul(SB[:, yh], one, mtr[64:65, sys_])

                usq = sb.tile([_FP, W], f32, tag="usq")
                nc.scalar.activation(usq[:], U[:], Act.Square)
                vs = sb.tile([_FP, W], f32, tag="vs")
                nc.scalar.activation(vs[:], V[:], Act.Copy)
                vsq = sb.tile([_FP, W], f32, tag="vsq")
                nc.scalar.activation(vsq[:], V[:], Act.Square)

                r1 = sb.tile([_FP, W], f32, tag="r1")
                nc.scalar.activation(r1[:], T[:], Act.Relu)
                kc1 = sb.tile([_FP, W], f32, tag="kc1")
                nc.vector.tensor_scalar(kc1[:], r1[:], _MAGIC,
                                        _MAGIC + 24.0, Op.add, Op.min)
                kc2 = sb.tile([_FP, W], f32, tag="kc2")
                nc.scalar.activation(kc2[:], kc1[:], Act.Copy, bias=-_MAGIC)
                sk = sb.tile([_FP, W], f32, tag="sk")
                nc.vector.tensor_tensor(sk[:], kc2[:], SB[:], Op.mult)
                val = sb.tile([_FP, W], f32, tag="val")
                nc.vector.tensor_tensor(val[:], U[:], sk[:], Op.subtract)
                vq = sb.tile([_FP, W], f32, tag="vq")
                nc.scalar.activation(vq[:], val[:], Act.Square)

                m1 = sb.tile([_FP, W], f32, tag="m1")
                nc.vector.tensor_tensor(m1[:], U[:], vs[:], Op.min)
                mm = sb.tile([_FP, _CHUNK], f32, tag="mm")
                nc.vector.tensor_tensor(mm[:], m1[:, xh], m1[:, yh], Op.min)

                exy = sb.tile([_FP, W], f32, tag="exy")
                nc.vector.tensor_tensor(exy[:], usq[:], vsq[:], Op.min)
                e1 = sb.tile([_FP, _CHUNK], f32, tag="e1")
                nc.vector.tensor_tensor(e1[:], exy[:, xh], vq[:, yh], Op.add)
                e2 = sb.tile([_FP, _CHUNK], f32, tag="e2")
                nc.vector.tensor_tensor(e2[:], exy[:, yh], vq[:, xh], Op.add)
                dmin = sb.tile([_FP, _CHUNK], f32, tag="dmin")
                nc.vector.tensor_tensor(dmin[:], e1[:], e2[:], Op.min)

                scr = sb.tile([_FP, _CHUNK], f32, tag="scr")
                nc.vector.scalar_tensor_tensor(
                    scr[:], mm[:], 0.0, dmin[:], Op.is_lt, Op.mult,
                    accum_out=part[:, c:c + 1])

            nc.sync.dma_start(out_d[:], part[:])
    nc.compile()
    return nc


def _combo_cols(A, T):
    """Per-row combo vectors (f32) for fragment-source boxes A vs grid
    boxes T.  Returns dict of [N] arrays."""
    A = A.astype(np.float32, copy=False)
    T = T.astype(np.float32, copy=False)
    w = A[:, 2] - A[:, 0]
    h = A[:, 3] - A[:, 1]
    tw = T[:, 2] - T[:, 0]
    th = T[:, 3] - T[:, 1]
    dx = A[:, 0] - T[:, 0]
    dy = A[:, 1] - T[:, 1]
    dvx = T[:, 2] - A[:, 0]
    dvy = T[:, 3] - A[:, 1]
    with np.errstate(divide="ignore"):
        rix = np.where(tw != 0, np.float32(24.0) / tw, np.float32(0.0))
        riy = np.where(th != 0, np.float32(24.0) / th, np.float32(0.0))
    rix = rix.astype(np.float32)
    riy = riy.astype(np.float32)
    return dict(
        w=w, dx=dx, nw=-w, dvx=dvx, wrx=w * rix, dxrx=dx * rix,
        h=h, dy=dy, nh=-h, dvy=dvy, wry=h * riy, dyry=dy * riy,
        sx=tw / np.float32(24.0), sy=th / np.float32(24.0))


def _mmin_for_core(boxes_c, targets_c):
    """Build the [6, _MMW] matmul-input DRAM tensor for one core."""
    d1 = _combo_cols(boxes_c, targets_c)     # dir1: cols 0:512
    d2 = _combo_cols(targets_c, boxes_c)     # dir2: cols 512:1024
    cat = {k: np.concatenate([d1[k], d2[k]]) for k in d1}

    m = np.zeros((6, _MMW), dtype=np.float32)
    m[0, :_FP] = np.repeat(_LIN10, 10)       # gx  (i of f = i*10+j)
    m[1, :_FP] = 1.0
    m[2, :_FP] = np.tile(_LIN10, 10)         # gy
    m[3, :_FP] = 1.0
    m[4, :_FP] = 1.0                         # ones (K=1 lhsT)
    B0, B1, B2 = (slice(_LW + b * _COLS, _LW + (b + 1) * _COLS)
                  for b in range(3))
    m[0, B0] = cat["w"];    m[1, B0] = cat["dx"]
    m[0, B1] = cat["nw"];   m[1, B1] = cat["dvx"]
    m[0, B2] = cat["wrx"];  m[1, B2] = cat["dxrx"]
    m[2, B0] = cat["h"];    m[3, B0] = cat["dy"]
    m[2, B1] = cat["nh"];   m[3, B1] = cat["dvy"]
    m[2, B2] = cat["wry"];  m[3, B2] = cat["dyry"]
    m[4, _LW:_LW + _COLS] = cat["sx"]
    m[4, _LW + _COLS:_LW + 2 * _COLS] = cat["sy"]
    return m


def kernel(boxes: np.ndarray, targets: np.ndarray) -> np.ndarray:
    from concourse.bass_utils import run_bass_kernel_spmd

    global LAST_RESULTS
    boxes = np.ascontiguousarray(boxes, dtype=np.float32)
    targets = np.ascontiguousarray(targets, dtype=np.float32)
    assert boxes.shape == (_B, 4) and targets.shape == (_B, 4)

    if "nc" not in _compiled:
        _compiled["nc"] = _build_nc()
    nc = _compiled["nc"]

    in_maps = []
    for c in range(_N_CORES):
        rows = slice(c * _BOX_PER_CORE, (c + 1) * _BOX_PER_CORE)
        in_maps.append({"mmin": _mmin_for_core(boxes[rows], targets[rows])})

    trace = bool(int(os.environ.get("BOXLOSS_TRACE", "0")))
    res = run_bass_kernel_spmd(nc, in_maps, list(range(_N_CORES)),
                               trace=trace)
    LAST_RESULTS = res

    total = np.float64(0.0)
    for r in res.results:
        total += r["out"].astype(np.float64).sum()
    loss = total / (2.0 * _B * _FP)
    return np.array(loss, dtype=np.float32)


# revision 23
# speedup vs baseline: 1.0391x; 1.0276x over previous
"""BoxRenderLoss Trainium2 kernel.

loss = mean over (box, fragment) pairs of masked min-squared-distance between
each box's 10x10 fragment grid and the other box's 100-point sampled boundary,
both directions, / (2*B*FP).

Algorithm: the min over the 100 boundary points decomposes into the 4 box
edges; each edge's 25-point uniform grid min has the closed form
k* = clamp(round(u/s), 0, 24), val = u - s*k*.  Per (row, fragment) item:
  dmin = min( min(ux^2, vx^2) + valy^2,  min(uy^2, vy^2) + valx^2 )
  mask = min(ux, vx, uy, vy) < 0         (fragment outside other box)
  contribution = dmin * mask

Device layout: partitions = 100 fragment points, free dim = virtual rows
(4096 boxes x 2 directions, data-parallel over 8 cores -> 1024 columns/core,
2 chunks of 512, x|y packed side by side -> FD 1024).  The affine maps
U = gx*w + dx, V = -gx*w + dvx, T = gx*(w*rix) + dx*rix and broadcast SB = s
are K<=2 outer-product matmuls on the TensorEngine.  All matmul operands live
in ONE SBUF tile (rows 0-1 / 32-33 / 64-65 for the base-partition-0/32/64
groups) and each PSUM tensor is consumed by exactly one engine -- PE Matmult
instructions only support a single semaphore wait.  Final per-partition row
sums come free via scalar_tensor_tensor's accum_out; host sums 100x2x8
partials and divides.
"""

import os
import numpy as np

# Exact float32 bit patterns of jnp.linspace(0.0, 1.0, 10) (fragment grid).
_LIN10 = np.array(
    [0, 1038323257, 1046711865, 1051372203, 1055100473,
     1057896676, 1059760811, 1061624946, 1063489081, 1065353216],
    dtype=np.uint32,
).view(np.float32)

_B = 4096
_FP = 100
_N_CORES = 8
_BOX_PER_CORE = _B // _N_CORES          # 512
_COLS = 2 * _BOX_PER_CORE               # 1024 virtual rows per core
_CHUNK = 512
_N_CHUNKS = _COLS // _CHUNK             # 2
_MAGIC = 8388608.0                      # 2^23 round-to-nearest trick

# mm-input tile column layout: [lhsT 0:128 | block0 | block1 | block2]
_LW = 128
_MMW = _LW + 3 * _COLS                  # 3200

LAST_RESULTS = None  # BassKernelResults of the most recent run (for test.py)

_compiled = {}


def _build_nc():
    import concourse.bass as bass
    import concourse.bacc as bacc
    import concourse.tile as tile
    from concourse import mybir

    f32 = mybir.dt.float32
    bf16 = mybir.dt.bfloat16
    Op = mybir.AluOpType
    Act = mybir.ActivationFunctionType

    nc = bacc.Bacc("TRN2", target_bir_lowering=False, debug=False,
                   num_devices=_N_CORES)
    f32r = mybir.dt.float32r
    mmin_d = nc.dram_tensor("mmin", [6, _MMW], f32r,
                            kind="ExternalInput").ap()
    out_d = nc.dram_tensor("out", [_FP, _N_CHUNKS], f32,
                           kind="ExternalOutput").ap()

    def blk(b, c):  # rhs slice columns for block b, chunk c
        start = _LW + b * _COLS + c * _CHUNK
        return slice(start, start + _CHUNK)

    from concourse.tile import add_dep_helper

    with tile.TileContext(nc) as tc:
        with (
            tc.tile_pool(name="const", bufs=1) as const,
            tc.tile_pool(name="sb", bufs=3) as sb,
            tc.tile_pool(name="ps", bufs=1, space="PSUM") as ps,
        ):
            mt = const.tile([66, _MMW], f32r)
            # One DMA per base-partition row group so every matmul carries at
            # most one new DMA wait.
            nc.sync.dma_start(mt[0:2, :], mmin_d[0:2, :])
            nc.sync.dma_start(mt[32:34, :], mmin_d[2:4, :])
            nc.sync.dma_start(mt[64:66, :], mmin_d[4:6, :])
            part = const.tile([_FP, _N_CHUNKS], f32)

            # float32r (same bits as f32): makes tile_legalize split each
            # Matmult into LdWeights + Matmult so semaphore waits spread
            # across two PE instructions (Matmult's LW slot fits only one).
            mtr = mt
            gx = mtr[0:2, :_FP]     # [gx; ones]
            gy = mtr[32:34, :_FP]   # [gy; ones]
            one = mtr[64:65, :_FP]  # [ones]

            for c in range(_N_CHUNKS):
                W = 2 * _CHUNK
                U = ps.tile([_FP, W], f32, tag="U")
                V = ps.tile([_FP, W], f32, tag="V")
                T = ps.tile([_FP, W], f32, tag="T")
                SB = ps.tile([_FP, W], f32, tag="SB")
                xh = slice(0, _CHUNK)
                yh = slice(_CHUNK, W)

                nc.tensor.matmul(U[:, xh], gx, mtr[0:2, blk(0, c)])
                nc.tensor.matmul(U[:, yh], gy, mtr[32:34, blk(0, c)])
                nc.tensor.matmul(V[:, xh], gx, mtr[0:2, blk(1, c)])
                nc.tensor.matmul(V[:, yh], gy, mtr[32:34, blk(1, c)])
                nc.tensor.matmul(T[:, xh], gx, mtr[0:2, blk(2, c)])
                nc.tensor.matmul(T[:, yh], gy, mtr[32:34, blk(2, c)])
                sxs = slice(_LW + c * _CHUNK, _LW + c * _CHUNK + _CHUNK)
                sys_ = slice(_LW + _COLS + c * _CHUNK,
                             _LW + _COLS + c * _CHUNK + _CHUNK)
                nc.tensor.matmul(SB[:, xh], one, mtr[64:65, sxs])
                nc.tensor.matmul(SB[:, yh], one, mtr[64:65, sys_])

                usq = sb.tile([_FP, W], bf16, tag="usq")
                nc.scalar.activation(usq[:], U[:], Act.Square)
                vs = sb.tile([_FP, W], f32, tag="vs")
                nc.scalar.activation(vs[:], V[:], Act.Copy)
                vsq = sb.tile([_FP, W], bf16, tag="vsq")
                nc.scalar.activation(vsq[:], V[:], Act.Square)

                r1 = sb.tile([_FP, W], f32, tag="r1")
                nc.scalar.activation(r1[:], T[:], Act.Relu)
                kc1 = sb.tile([_FP, W], f32, tag="kc1")
                nc.vector.tensor_scalar(kc1[:], r1[:], _MAGIC,
                                        _MAGIC + 24.0, Op.add, Op.min)
                kc2 = sb.tile([_FP, W], f32, tag="kc2")
                nc.scalar.activation(kc2[:], kc1[:], Act.Copy, bias=-_MAGIC)
                sk = sb.tile([_FP, W], f32, tag="sk")
                nc.vector.tensor_tensor(sk[:], kc2[:], SB[:], Op.mult)
                val = sb.tile([_FP, W], f32, tag="val")
                nc.vector.tensor_tensor(val[:], U[:], sk[:], Op.subtract)
                vq = sb.tile([_FP, W], bf16, tag="vq")
                nc.scalar.activation(vq[:], val[:], Act.Square)

                m1 = sb.tile([_FP, W], f32, tag="m1")
                nc.vector.tensor_tensor(m1[:], U[:], vs[:], Op.min)
                mm = sb.tile([_FP, _CHUNK], f32, tag="mm")
                nc.vector.tensor_tensor(mm[:], m1[:, xh], m1[:, yh], Op.min)

                exy = sb.tile([_FP, W], bf16, tag="exy")
                nc.vector.tensor_tensor(exy[:], usq[:], vsq[:], Op.min)
                e1 = sb.tile([_FP, _CHUNK], bf16, tag="e1")
                nc.vector.tensor_tensor(e1[:], exy[:, xh], vq[:, yh], Op.add)
                e2 = sb.tile([_FP, _CHUNK], bf16, tag="e2")
                nc.vector.tensor_tensor(e2[:], exy[:, yh], vq[:, xh], Op.add)
                dmin = sb.tile([_FP, _CHUNK], f32, tag="dmin")
                nc.vector.tensor_tensor(dmin[:], e1[:], e2[:], Op.min)

                scr = sb.tile([_FP, _CHUNK], f32, tag="scr")
                nc.vector.scalar_tensor_tensor(
                    scr[:], mm[:], 0.0, dmin[:], Op.is_lt, Op.mult,
                    accum_out=part[:, c:c + 1])

            nc.sync.dma_start(out_d[:], part[:])
    nc.compile()
    return nc


def _combo_cols(A, T):
    """Per-row combo vectors (f32) for fragment-source boxes A vs grid
    boxes T.  Returns dict of [N] arrays."""
    A = A.astype(np.float32, copy=False)
    T = T.astype(np.float32, copy=False)
    w = A[:, 2] - A[:, 0]
    h = A[:, 3] - A[:, 1]
    tw = T[:, 2] - T[:, 0]
    th = T[:, 3] - T[:, 1]
    dx = A[:, 0] - T[:, 0]
    dy = A[:, 1] - T[:, 1]
    dvx = T[:, 2] - A[:, 0]
    dvy = T[:, 3] - A[:, 1]
    with np.errstate(divide="ignore"):
        rix = np.where(tw != 0, np.float32(24.0) / tw, np.float32(0.0))
        riy = np.where(th != 0, np.float32(24.0) / th, np.float32(0.0))
    rix = rix.astype(np.float32)
    riy = riy.astype(np.float32)
    return dict(
        w=w, dx=dx, nw=-w, dvx=dvx, wrx=w * rix, dxrx=dx * rix,
        h=h, dy=dy, nh=-h, dvy=dvy, wry=h * riy, dyry=dy * riy,
        sx=tw / np.float32(24.0), sy=th / np.float32(24.0))


def _mmin_for_core(boxes_c, targets_c):
    """Build the [6, _MMW] matmul-input DRAM tensor for one core."""
    d1 = _combo_cols(boxes_c, targets_c)     # dir1: cols 0:512
    d2 = _combo_cols(targets_c, boxes_c)     # dir2: cols 512:1024
    cat = {k: np.concatenate([d1[k], d2[k]]) for k in d1}

    m = np.zeros((6, _MMW), dtype=np.float32)
    m[0, :_FP] = np.repeat(_LIN10, 10)       # gx  (i of f = i*10+j)
    m[1, :_FP] = 1.0
    m[2, :_FP] = np.tile(_LIN10, 10)         # gy
    m[3, :_FP] = 1.0
    m[4, :_FP] = 1.0                         # ones (K=1 lhsT)
    B0, B1, B2 = (slice(_LW + b * _COLS, _LW + (b + 1) * _COLS)
                  for b in range(3))
    m[0, B0] = cat["w"];    m[1, B0] = cat["dx"]
    m[0, B1] = cat["nw"];   m[1, B1] = cat["dvx"]
    m[0, B2] = cat["wrx"];  m[1, B2] = cat["dxrx"]
    m[2, B0] = cat["h"];    m[3, B0] = cat["dy"]
    m[2, B1] = cat["nh"];   m[3, B1] = cat["dvy"]
    m[2, B2] = cat["wry"];  m[3, B2] = cat["dyry"]
    m[4, _LW:_LW + _COLS] = cat["sx"]
    m[4, _LW + _COLS:_LW + 2 * _COLS] = cat["sy"]
    return m


def kernel(boxes: np.ndarray, targets: np.ndarray) -> np.ndarray:
    from concourse.bass_utils import run_bass_kernel_spmd

    global LAST_RESULTS
    boxes = np.ascontiguousarray(boxes, dtype=np.float32)
    targets = np.ascontiguousarray(targets, dtype=np.float32)
    assert boxes.shape == (_B, 4) and targets.shape == (_B, 4)

    if "nc" not in _compiled:
        _compiled["nc"] = _build_nc()
    nc = _compiled["nc"]

    in_maps = []
    for c in range(_N_CORES):
        rows = slice(c * _BOX_PER_CORE, (c + 1) * _BOX_PER_CORE)
        in_maps.append({"mmin": _mmin_for_core(boxes[rows], targets[rows])})

    trace = bool(int(os.environ.get("BOXLOSS_TRACE", "0")))
    res = run_bass_kernel_spmd(nc, in_maps, list(range(_N_CORES)),
                               trace=trace)
    LAST_RESULTS = res

    total = np.float64(0.0)
    for r in res.results:
        total += r["out"].astype(np.float64).sum()
    loss = total / (2.0 * _B * _FP)
    return np.array(loss, dtype=np.float32)


# revision 24
# speedup vs baseline: 1.0694x; 1.0291x over previous
"""BoxRenderLoss Trainium2 kernel.

loss = mean over (box, fragment) pairs of masked min-squared-distance between
each box's 10x10 fragment grid and the other box's 100-point sampled boundary,
both directions, / (2*B*FP).

Algorithm: the min over the 100 boundary points decomposes into the 4 box
edges; each edge's 25-point uniform grid min has the closed form
k* = clamp(round(u/s), 0, 24), val = u - s*k*.  Per (row, fragment) item:
  dmin = min( min(ux^2, vx^2) + valy^2,  min(uy^2, vy^2) + valx^2 )
  mask = min(ux, vx, uy, vy) < 0         (fragment outside other box)
  contribution = dmin * mask

Device layout: partitions = 100 fragment points, free dim = virtual rows
(4096 boxes x 2 directions, data-parallel over 8 cores -> 1024 columns/core,
2 chunks of 512, x|y packed side by side -> FD 1024).  The affine maps
U = gx*w + dx, V = -gx*w + dvx, T = gx*(w*rix) + dx*rix and broadcast SB = s
are K<=2 outer-product matmuls on the TensorEngine.  All matmul operands live
in ONE SBUF tile (rows 0-1 / 32-33 / 64-65 for the base-partition-0/32/64
groups) and each PSUM tensor is consumed by exactly one engine -- PE Matmult
instructions only support a single semaphore wait.  Final per-partition row
sums come free via scalar_tensor_tensor's accum_out; host sums 100x2x8
partials and divides.
"""

import os
import numpy as np

# Exact float32 bit patterns of jnp.linspace(0.0, 1.0, 10) (fragment grid).
_LIN10 = np.array(
    [0, 1038323257, 1046711865, 1051372203, 1055100473,
     1057896676, 1059760811, 1061624946, 1063489081, 1065353216],
    dtype=np.uint32,
).view(np.float32)

_B = 4096
_FP = 100
_N_CORES = 8
_BOX_PER_CORE = _B // _N_CORES          # 512
_COLS = 2 * _BOX_PER_CORE               # 1024 virtual rows per core
_CHUNK = 512
_N_CHUNKS = _COLS // _CHUNK             # 2
_MAGIC = 8388608.0                      # 2^23 round-to-nearest trick

# mm-input tile column layout: [lhsT 0:128 | block0 | block1 | block2]
_LW = 128
_MMW = _LW + 3 * _COLS                  # 3200

LAST_RESULTS = None  # BassKernelResults of the most recent run (for test.py)

_compiled = {}


def _build_nc():
    import concourse.bass as bass
    import concourse.bacc as bacc
    import concourse.tile as tile
    from concourse import mybir

    f32 = mybir.dt.float32
    bf16 = mybir.dt.bfloat16
    Op = mybir.AluOpType
    Act = mybir.ActivationFunctionType

    nc = bacc.Bacc("TRN2", target_bir_lowering=False, debug=False,
                   num_devices=_N_CORES)
    f32r = mybir.dt.float32r
    mmin_d = nc.dram_tensor("mmin", [6, _MMW], f32r,
                            kind="ExternalInput").ap()
    out_d = nc.dram_tensor("out", [_FP, _N_CHUNKS], f32,
                           kind="ExternalOutput").ap()

    def blk(b, c):  # rhs slice columns for block b, chunk c
        start = _LW + b * _COLS + c * _CHUNK
        return slice(start, start + _CHUNK)

    from concourse.tile import add_dep_helper

    with tile.TileContext(nc) as tc:
        with (
            tc.tile_pool(name="const", bufs=1) as const,
            tc.tile_pool(name="sb", bufs=4) as sb,
            tc.tile_pool(name="ps", bufs=1, space="PSUM") as ps,
        ):
            mt = const.tile([66, _MMW], f32r)
            # One DMA per base-partition row group so every matmul carries at
            # most one new DMA wait.
            nc.sync.dma_start(mt[0:2, :], mmin_d[0:2, :])
            nc.sync.dma_start(mt[32:34, :], mmin_d[2:4, :])
            nc.sync.dma_start(mt[64:66, :], mmin_d[4:6, :])
            part = const.tile([_FP, _N_CHUNKS], f32)

            # float32r (same bits as f32): makes tile_legalize split each
            # Matmult into LdWeights + Matmult so semaphore waits spread
            # across two PE instructions (Matmult's LW slot fits only one).
            mtr = mt
            gx = mtr[0:2, :_FP]     # [gx; ones]
            gy = mtr[32:34, :_FP]   # [gy; ones]
            one = mtr[64:65, :_FP]  # [ones]

            for c in range(_N_CHUNKS):
                W = 2 * _CHUNK
                U = ps.tile([_FP, W], f32, tag="U")
                V = ps.tile([_FP, W], f32, tag="V")
                T = ps.tile([_FP, W], f32, tag="T")
                SB = ps.tile([_FP, W], f32, tag="SB")
                xh = slice(0, _CHUNK)
                yh = slice(_CHUNK, W)

                nc.tensor.matmul(U[:, xh], gx, mtr[0:2, blk(0, c)])
                nc.tensor.matmul(U[:, yh], gy, mtr[32:34, blk(0, c)])
                nc.tensor.matmul(V[:, xh], gx, mtr[0:2, blk(1, c)])
                nc.tensor.matmul(V[:, yh], gy, mtr[32:34, blk(1, c)])
                nc.tensor.matmul(T[:, xh], gx, mtr[0:2, blk(2, c)])
                nc.tensor.matmul(T[:, yh], gy, mtr[32:34, blk(2, c)])
                sxs = slice(_LW + c * _CHUNK, _LW + c * _CHUNK + _CHUNK)
                sys_ = slice(_LW + _COLS + c * _CHUNK,
                             _LW + _COLS + c * _CHUNK + _CHUNK)
                nc.tensor.matmul(SB[:, xh], one, mtr[64:65, sxs])
                nc.tensor.matmul(SB[:, yh], one, mtr[64:65, sys_])

                usq = sb.tile([_FP, W], bf16, tag="usq")
                nc.scalar.activation(usq[:], U[:], Act.Square)
                vs = sb.tile([_FP, W], f32, tag="vs")
                nc.scalar.activation(vs[:], V[:], Act.Copy)
                vsq = sb.tile([_FP, W], bf16, tag="vsq")
                nc.scalar.activation(vsq[:], V[:], Act.Square)

                r1 = sb.tile([_FP, W], f32, tag="r1")
                nc.scalar.activation(r1[:], T[:], Act.Relu)
                kc1 = sb.tile([_FP, W], f32, tag="kc1")
                nc.vector.tensor_scalar(kc1[:], r1[:], _MAGIC,
                                        _MAGIC + 24.0, Op.add, Op.min)
                sk = sb.tile([_FP, W], f32, tag="sk")
                nc.vector.scalar_tensor_tensor(sk[:], kc1[:], _MAGIC, SB[:],
                                               Op.subtract, Op.mult)
                val = sb.tile([_FP, W], f32, tag="val")
                nc.vector.tensor_tensor(val[:], U[:], sk[:], Op.subtract)
                vq = sb.tile([_FP, W], bf16, tag="vq")
                nc.scalar.activation(vq[:], val[:], Act.Square)

                m1 = sb.tile([_FP, W], f32, tag="m1")
                nc.vector.tensor_tensor(m1[:], U[:], vs[:], Op.min)
                mm = sb.tile([_FP, _CHUNK], f32, tag="mm")
                nc.vector.tensor_tensor(mm[:], m1[:, xh], m1[:, yh], Op.min)

                exy = sb.tile([_FP, W], bf16, tag="exy")
                nc.vector.tensor_tensor(exy[:], usq[:], vsq[:], Op.min)
                e1 = sb.tile([_FP, _CHUNK], bf16, tag="e1")
                nc.vector.tensor_tensor(e1[:], exy[:, xh], vq[:, yh], Op.add)
                e2 = sb.tile([_FP, _CHUNK], bf16, tag="e2")
                nc.vector.tensor_tensor(e2[:], exy[:, yh], vq[:, xh], Op.add)
                dmin = sb.tile([_FP, _CHUNK], f32, tag="dmin")
                nc.vector.tensor_tensor(dmin[:], e1[:], e2[:], Op.min)

                scr = sb.tile([_FP, _CHUNK], f32, tag="scr")
                nc.vector.scalar_tensor_tensor(
                    scr[:], mm[:], 0.0, dmin[:], Op.is_lt, Op.mult,
                    accum_out=part[:, c:c + 1])

            nc.sync.dma_start(out_d[:], part[:])
    nc.compile()
    return nc


def _combo_cols(A, T):
    """Per-row combo vectors (f32) for fragment-source boxes A vs grid
    boxes T.  Returns dict of [N] arrays."""
    A = A.astype(np.float32, copy=False)
    T = T.astype(np.float32, copy=False)
    w = A[:, 2] - A[:, 0]
    h = A[:, 3] - A[:, 1]
    tw = T[:, 2] - T[:, 0]
    th = T[:, 3] - T[:, 1]
    dx = A[:, 0] - T[:, 0]
    dy = A[:, 1] - T[:, 1]
    dvx = T[:, 2] - A[:, 0]
    dvy = T[:, 3] - A[:, 1]
    with np.errstate(divide="ignore"):
        rix = np.where(tw != 0, np.float32(24.0) / tw, np.float32(0.0))
        riy = np.where(th != 0, np.float32(24.0) / th, np.float32(0.0))
    rix = rix.astype(np.float32)
    riy = riy.astype(np.float32)
    return dict(
        w=w, dx=dx, nw=-w, dvx=dvx, wrx=w * rix, dxrx=dx * rix,
        h=h, dy=dy, nh=-h, dvy=dvy, wry=h * riy, dyry=dy * riy,
        sx=tw / np.float32(24.0), sy=th / np.float32(24.0))


def _mmin_for_core(boxes_c, targets_c):
    """Build the [6, _MMW] matmul-input DRAM tensor for one core."""
    d1 = _combo_cols(boxes_c, targets_c)     # dir1: cols 0:512
    d2 = _combo_cols(targets_c, boxes_c)     # dir2: cols 512:1024
    cat = {k: np.concatenate([d1[k], d2[k]]) for k in d1}

    m = np.zeros((6, _MMW), dtype=np.float32)
    m[0, :_FP] = np.repeat(_LIN10, 10)       # gx  (i of f = i*10+j)
    m[1, :_FP] = 1.0
    m[2, :_FP] = np.tile(_LIN10, 10)         # gy
    m[3, :_FP] = 1.0
    m[4, :_FP] = 1.0                         # ones (K=1 lhsT)
    B0, B1, B2 = (slice(_LW + b * _COLS, _LW + (b + 1) * _COLS)
                  for b in range(3))
    m[0, B0] = cat["w"];    m[1, B0] = cat["dx"]
    m[0, B1] = cat["nw"];   m[1, B1] = cat["dvx"]
    m[0, B2] = cat["wrx"];  m[1, B2] = cat["dxrx"]
    m[2, B0] = cat["h"];    m[3, B0] = cat["dy"]
    m[2, B1] = cat["nh"];   m[3, B1] = cat["dvy"]
    m[2, B2] = cat["wry"];  m[3, B2] = cat["dyry"]
    m[4, _LW:_LW + _COLS] = cat["sx"]
    m[4, _LW + _COLS:_LW + 2 * _COLS] = cat["sy"]
    return m


def kernel(boxes: np.ndarray, targets: np.ndarray) -> np.ndarray:
    from concourse.bass_utils import run_bass_kernel_spmd

    global LAST_RESULTS
    boxes = np.ascontiguousarray(boxes, dtype=np.float32)
    targets = np.ascontiguousarray(targets, dtype=np.float32)
    assert boxes.shape == (_B, 4) and targets.shape == (_B, 4)

    if "nc" not in _compiled:
        _compiled["nc"] = _build_nc()
    nc = _compiled["nc"]

    in_maps = []
    for c in range(_N_CORES):
        rows = slice(c * _BOX_PER_CORE, (c + 1) * _BOX_PER_CORE)
        in_maps.append({"mmin": _mmin_for_core(boxes[rows], targets[rows])})

    trace = bool(int(os.environ.get("BOXLOSS_TRACE", "0")))
    res = run_bass_kernel_spmd(nc, in_maps, list(range(_N_CORES)),
                               trace=trace)
    LAST_RESULTS = res

    total = np.float64(0.0)
    for r in res.results:
        total += r["out"].astype(np.float64).sum()
    loss = total / (2.0 * _B * _FP)
    return np.array(loss, dtype=np.float32)
